# revision 54
# baseline (speedup 1.0000x reference)
"""Causal multi-head attention (B=2, S=2048, D=1024, H=16, hd=64) on 8 trn2 cores.

Sharding: core c handles batch b = c//4 and head group hg = c%4 (4 heads each).
Each core computes its Q/K/V shard (tensor-parallel columns of W_qkv), causal
attention for its 4 heads with scores held transposed ([s_k, s_q] so the PV
matmul needs no on-chip transposes), and a partial output projection over its
256 rows of W_proj. The host sums the 4 partials per batch and adds the exact
bias terms (softmax rows sum to 1, so attn@(V + 1 bv^T) = attn@V + bv^T; the
b_qkv V-slice and b_proj are applied on the host).

v3 schedule:
- inputs stream in k-chunks ordered by first use (qk-f0 weights + x half 0
  first); the host packs W_qkv columns as [qk_f0 | wv | qk_f1] per core so the
  critical prefix is contiguous and minimal.
- dummy matmuls (prelude) and standalone LDWEIGHTS (filler-thin attention
  iterations) keep the PE HAM clock gate at 8/8 so nothing runs at half clock.
- attention pairs software-pipeline the PV matmuls one ki behind scores/exp:
  the next exp never queues behind a PV waiting on the current exp.
- PE filler work (QKV f1 projections, V groups, half-width output-projection
  groups) is spread across iterations at <=1.7us granularity, weighted toward
  the late pairs.
- finishes (softmax normalize) are split by column range and emitted as soon
  as the psum columns stop accumulating, so the output projection of finished
  q-tiles overlaps the last attention pair and the tail is ~6us.
- proj results are cast to bf16 (host accumulates in f32), halving out-DMA.
"""

import numpy as np
import ml_dtypes
from contextlib import ExitStack
from functools import partial

B, S, D, H = 2, 2048, 1024, 16
HD = 64
NCORES = 8
FPC = 256  # features per core (4 heads x 64)

_CACHE = {}


def _build():
    import concourse.bacc as bacc
    import concourse.tile as tile
    import concourse.mybir as mybir

    f32 = mybir.dt.float32
    bf16 = mybir.dt.bfloat16

    nc = bacc.Bacc("TRN2", target_bir_lowering=False, debug=False, num_devices=NCORES)

    xT = nc.dram_tensor("xT", [D, S], bf16, kind="ExternalInput").ap()
    wqkv = nc.dram_tensor("wqkv", [D, 3 * FPC], bf16, kind="ExternalInput").ap()
    wp = nc.dram_tensor("wp", [FPC, D], bf16, kind="ExternalInput").ap()
    bqk = nc.dram_tensor("bqk", [128, 4], f32, kind="ExternalInput").ap()
    out = nc.dram_tensor("out", [S, D], bf16, kind="ExternalOutput").ap()

    with tile.TileContext(nc) as tc:
        with ExitStack() as ctx:
            _body(ctx, tc, mybir, out, xT, wqkv, wp, bqk)

    nc.compile()
    return nc


def _body(ctx, tc, mybir, out, xT, wqkv, wp, bqk):
    nc = tc.nc
    f32 = mybir.dt.float32
    bf16 = mybir.dt.bfloat16
    AF = mybir.ActivationFunctionType
    NK = D // 128   # 8 contraction tiles for qkv/proj-input dim
    NS = S // 128   # 16 sequence tiles

    sb = ctx.enter_context(tc.tile_pool(name="sb", bufs=1))

    xt_a = sb.tile([128, NK * S], bf16, name="xta", tag="xta")
    xt_t = [xt_a[:, k * S:(k + 1) * S] for k in range(NK)]
    # per k-block: [wq_f0 wk_f0 (256) | wq_f1 wk_f1 (256) | wv (256)]
    w_a = sb.tile([128, NK * 3 * FPC], bf16, name="wa", tag="wa")

    def wq_f(k, f):
        o = k * 768 + f * 256
        return w_a[:, o:o + 128]

    def wk_f(k, f):
        o = k * 768 + f * 256 + 128
        return w_a[:, o:o + 128]

    wv_t = [w_a[:, k * 768 + 512: k * 768 + 768] for k in range(NK)]
    wp_a = sb.tile([128, 2 * D], bf16, name="wpa", tag="wpa")
    wp_t = [wp_a[:, k * D:(k + 1) * D] for k in range(2)]
    qt_t = [sb.tile([128, S], bf16, name=f"qtt{f}", tag=f"qtt{f}") for f in range(2)]
    kt_t = [sb.tile([128, S], bf16, name=f"ktt{f}", tag=f"ktt{f}") for f in range(2)]
    v_t = [sb.tile([128, 4 * 65], bf16, name=f"vt{s}", tag=f"vt{s}") for s in range(NS)]
    ot_t = [sb.tile([128, S], bf16, name=f"ott{f}", tag=f"ott{f}") for f in range(2)]
    bqk_t = sb.tile([128, 4], f32, name="bqkt", tag="bqkt")
    mask_t = sb.tile([128, 128], bf16, name="maskt", tag="maskt")

    p_pool = ctx.enter_context(tc.tile_pool(name="pp", bufs=6))
    rc_pool = ctx.enter_context(tc.tile_pool(name="rcp", bufs=2))
    oo_pool = ctx.enter_context(tc.tile_pool(name="oop", bufs=4))

    # causal keep-mask for the transposed diag block (keep q >= k), built
    # on-device: GPSIMD is idle from t~2.5us while every DMA pays multi-us
    # descriptor/issue latency. The HAM warm-up dummies read mask_t, so this
    # also unblocks them ~6us earlier than a mask DMA would.
    nc.gpsimd.memset(mask_t[:], 1.0)
    nc.gpsimd.affine_select(
        out=mask_t[:], in_=mask_t[:], compare_op=mybir.AluOpType.is_ge,
        fill=0.0, base=0, pattern=[[1, 128]], channel_multiplier=-1,
    )

    # ---- input DMAs. Each dma_start costs ~1us+ of SP issue time, and all
    # in-flight transfers SHARE the HBM bandwidth (so a late transfer steals
    # from an urgent one). The critical prefix (qk-f0 weights, x half 0, wv)
    # is issued immediately; the rest is gated on prelude progress via tiny
    # memset WAW anchors emitted later in program order.
    nc.sync.dma_start(bqk_t[:], bqk[:])
    w4 = wqkv.rearrange("(k p) (g c) -> p k g c", p=128, g=3)
    wa4 = w_a.rearrange("p (k g c) -> p k g c", k=NK, g=3)
    x3 = xT.rearrange("(k p) s -> p k s", p=128)
    xa3 = xt_a.rearrange("p (k s) -> p k s", k=NK)
    nc.sync.dma_start(wa4[:, 0:4, 0:1, :], w4[:, 0:4, 0:1, :])  # qk f0 k0-3
    nc.sync.dma_start(wa4[:, 4:8, 0:1, :], w4[:, 4:8, 0:1, :])  # qk f0 k4-7
    nc.sync.dma_start(xa3[:, 0:4, 0:1024], x3[:, 0:4, 0:1024])
    nc.sync.dma_start(xa3[:, 4:8, 0:1024], x3[:, 4:8, 0:1024])
    nc.sync.dma_start(wa4[:, :, 2:3, :], w4[:, :, 2:3, :])      # wv

    def dma_wf1():
        nc.vector.memset(w_a[:, 256:257], 0.0)
        nc.sync.dma_start(wa4[:, :, 1:2, :], w4[:, :, 1:2, :])  # qk f1

    def dma_xc21():
        nc.vector.memset(xt_a[:, 1024:1025], 0.0)
        nc.vector.memset(xt_a[:, 4 * S + 1024:4 * S + 1025], 0.0)
        nc.sync.dma_start(xa3[:, 0:4, 1024:2048], x3[:, 0:4, 1024:2048])
        nc.sync.dma_start(xa3[:, 4:8, 1024:2048], x3[:, 4:8, 1024:2048])

    def dma_wp():
        nc.vector.memset(wp_a[:, 0:1], 0.0)
        nc.sync.dma_start(wp_a.rearrange("p (k f) -> p k f", k=2),
                          wp.rearrange("(k p) f -> p k f", p=128))

    # PSUM (8 banks): "sc" slots 2 banks x2 (scores + filler rotation),
    # "pv" 2 banks x2.
    scp = ctx.enter_context(tc.tile_pool(name="ps_sc", bufs=2, space="PSUM"))
    pvp = ctx.enter_context(tc.tile_pool(name="ps_pv", bufs=2, space="PSUM"))

    # ---- HAM warmup. warm(): dummy matmuls into a borrowed pv-pool slot
    # (prelude only; retired long before the first attention pair).
    # lw(): standalone LDWEIGHTS — PE-busy, touches no psum — used to pad
    # filler-thin attention iterations so the clock gate never closes.
    def warm(n):
        # rotate over 7 disjoint column ranges: consecutive dummies have no
        # WAW dependency and issue back-to-back instead of ~335ns apart
        warm_t = pvp.tile([128, 1024], f32, name="pv", tag="pv", bufs=2)
        for i in range(n):
            c = (i % 7) * 128
            nc.tensor.matmul(warm_t[:, c:c + 128], mask_t[:], mask_t[:],
                             start=True, stop=True)

    def lw(n):
        def f():
            for _ in range(n):
                nc.tensor.ldweights(mask_t[:])
        return f

    def paw(units, n):
        """PE-array warmers for filler-thin attention iterations: real
        matmuls into the UNUSED partitions 65..127 of the live pv tile.
        start=False avoids the bank-wide has_written clear, so the live
        accumulation in rows 0..64 is untouched (disjoint elements)."""
        def f():
            pv = units[0].pv
            for i in range(n):
                c = (i % 7) * 128
                nc.tensor.matmul(pv[96:128, c:c + 128], mask_t[:, 0:32],
                                 mask_t[:], start=False, stop=True,
                                 skip_group_check=True, tile_position=(0, 96))
        return f

    def qkt_sp(dst, wsel, bcol, f, c2, sp, interleave=0):
        """Half (512 q cols) of one [128,1024] Q^T/K^T projection group."""
        wf = wq_f if wsel == "q" else wk_f
        ps = scp.tile([128, 1024], f32, name="sc", tag="sc", bufs=2)
        for k in range(NK):
            if interleave:
                warm(interleave)
            nc.tensor.matmul(
                ps[:, 0:512],
                wf(k, f),
                xt_t[k][:, c2 * 1024 + sp * 512: c2 * 1024 + (sp + 1) * 512],
                start=(k == 0), stop=(k == NK - 1),
            )
        nc.vector.tensor_scalar_add(
            dst[f][:, c2 * 1024 + sp * 512:c2 * 1024 + (sp + 1) * 512],
            ps[:, 0:512],
            bqk_t[:, bcol + f: bcol + f + 1],
        )

    def v_group(s):
        psv = scp.tile([128, 1024], f32, name="sc", tag="sc", bufs=2)
        for k in range(NK):
            nc.tensor.matmul(
                psv[:, 0:FPC],
                xt_t[k][:, s * 128:(s + 1) * 128],
                wv_t[k][:],
                start=(k == 0), stop=(k == NK - 1),
            )
        v3 = v_t[s].rearrange("p (h c) -> p h c", h=4)
        nc.vector.tensor_copy(v3[:, :, 0:64],
                              psv[:, 0:FPC].rearrange("p (h c) -> p h c", h=4))
        nc.vector.memset(v3[:, :, 64:65], 1.0)

    class AttnUnit:
        """Causal attention for head h over queries [half*1024, +1024)."""

        def __init__(self, h, half):
            self.h, self.half = h, half
            self.hp, self.hh = h // 2, h % 2
            self.r0 = self.hh * 64
            self.q0 = half * 1024
            self.ki_n = NS // 2 * (half + 1)
            self.pv = pvp.tile([128, 1024], f32, name="pv", tag="pv", bufs=2)
            self.P = {}

        def a0(self, ki):
            return max(ki * 128 - self.q0, 0)

        def spans(self, ki):
            a0 = self.a0(ki)
            return [(a0, 512), (512, 1024)] if a0 < 512 else [(a0, 1024)]

        def emit_scores(self, ki):
            q0, r0 = self.q0, self.r0
            qt, kt = qt_t[self.hp], kt_t[self.hp]
            self.sc = scp.tile([128, 1024], f32, name="sc", tag="sc", bufs=2)
            for (a, b) in self.spans(ki):
                nc.tensor.matmul(
                    self.sc[:, a:b],
                    kt[r0:r0 + 64, ki * 128:(ki + 1) * 128],
                    qt[r0:r0 + 64, q0 + a:q0 + b],
                    start=True, stop=True,
                )

        def emit_exp(self, ki):
            a0 = self.a0(ki)
            P = p_pool.tile([128, 1024], bf16, name="P", tag="P", bufs=6)
            self.P[ki] = P
            nc.scalar.activation(P[:, a0:1024], self.sc[:, a0:1024], AF.Exp,
                                 scale=float(HD) ** -0.5)
            if ki * 128 >= self.q0:  # causal mask on the diagonal block
                nc.vector.tensor_mul(P[:, a0:a0 + 128],
                                     P[:, a0:a0 + 128], mask_t[:])

        def emit_pv(self, ki):
            P = self.P.pop(ki)
            for (a, b) in self.spans(ki):
                last_ki = min(self.ki_n - 1, (self.q0 + b - 1) // 128)
                nc.tensor.matmul(
                    self.pv[0:65, a:b],
                    v_t[ki][:, self.h * 65:self.h * 65 + 65],
                    P[:, a:b],
                    start=(ki == 0), stop=(ki == last_ki),
                )

        def finish_cols(self, c0, c1):
            """Normalize q columns [q0+c0, q0+c1) once their pv is final."""
            pv = self.pv
            n = c1 - c0
            dcp = rc_pool.tile([1, 512], f32, name="dcp", tag="dcp", bufs=4)
            nc.vector.tensor_copy(dcp[:, 0:n], pv[64:65, c0:c1])
            rcp = rc_pool.tile([1, 512], f32, name="rcp", tag="rcp", bufs=4)
            nc.vector.reciprocal_approx_fast(rcp[:, 0:n], dcp[:, 0:n])
            rbc = rc_pool.tile([64, 512], f32, name="rbc", tag="rbc", bufs=4)
            nc.gpsimd.partition_broadcast(rbc[:, 0:n], rcp[:, 0:n], channels=64)
            nc.vector.tensor_mul(
                ot_t[self.hp][self.r0:self.r0 + 64,
                              self.q0 + c0:self.q0 + c1],
                pv[0:64, c0:c1], rbc[:, 0:n],
            )

    def attn_pair(units, fillers=(), head=None, start_ki=0):
        """Two heads interleaved at ki granularity. PV runs one ki behind
        scores/exp so the ACT engine never waits on a PV head-of-line stall.
        fillers[ki] thunks are emitted between exp(ki) and pv(ki-1).
        head=(next pair's units): their ki-0 scores/exp are emitted BEFORE
        this pair's trailing PVs so the ACT chain never drains across the
        pair boundary; the next attn_pair call passes start_ki=1 (the unit
        objects carry their ki-0 state over)."""
        ua, ub = units
        n = ua.ki_n
        for ki in range(start_ki, n):
            # adjacent scores land in different PE row groups (heads at
            # partition 0 and 64) and execute concurrently in the array
            ua.emit_scores(ki)
            ub.emit_scores(ki)
            ua.emit_exp(ki)
            ub.emit_exp(ki)
            if ki < len(fillers):
                for fn in fillers[ki]:
                    fn()
            if ki > 0:
                ua.emit_pv(ki - 1)
                ub.emit_pv(ki - 1)
        if head is not None:
            head[0].emit_scores(0)
            head[1].emit_scores(0)
            head[0].emit_exp(0)
            head[1].emit_exp(0)
        ua.emit_pv(n - 1)
        ub.emit_pv(n - 1)
        return units

    def proj_h(s, nh):
        """Half-width (512 features) output-projection of q-tile s."""
        pj = scp.tile([128, 1024], f32, name="sc", tag="sc", bufs=2)
        for k2 in range(2):
            nc.tensor.matmul(
                pj[:, 0:512],
                ot_t[k2][:, s * 128:(s + 1) * 128],
                wp_t[k2][:, nh * 512:(nh + 1) * 512],
                start=(k2 == 0), stop=(k2 == 1),
            )
        oo = oo_pool.tile([128, 512], bf16, name="oo", tag="oo", bufs=4)
        nc.vector.tensor_copy(oo[:], pj[:, 0:512])
        nc.sync.dma_start(out[s * 128:(s + 1) * 128, nh * 512:(nh + 1) * 512],
                          oo[:])

    def ph(s):
        return [partial(proj_h, s, 0), partial(proj_h, s, 1)]

    def proj_full(s):
        """Full-width projection of q-tile s with a single out-DMA (tail:
        fewer dma_starts, each ~1.2us of SP issue time). The evacuation copy
        is split across VectorE and the (post-attention idle) ScalarE so the
        two halves move in parallel."""
        pj = scp.tile([128, 1024], f32, name="sc", tag="sc", bufs=2)
        for nh in range(2):
            for k2 in range(2):
                nc.tensor.matmul(
                    pj[:, nh * 512:(nh + 1) * 512],
                    ot_t[k2][:, s * 128:(s + 1) * 128],
                    wp_t[k2][:, nh * 512:(nh + 1) * 512],
                    start=(k2 == 0), stop=(k2 == 1),
                )
        oo = oo_pool.tile([128, 1024], bf16, name="oof", tag="oof", bufs=2)
        nc.vector.tensor_copy(oo[:, 0:512], pj[:, 0:512])
        nc.scalar.copy(oo[:, 512:1024], pj[:, 512:1024])
        nc.sync.dma_start(out[s * 128:(s + 1) * 128, :], oo[:])

    def fin(units, c0, c1):
        def f():
            for u in units:
                u.finish_cols(c0, c1)
        return f

    q_sp = partial(qkt_sp, qt_t, "q", 0)
    k_sp = partial(qkt_sp, kt_t, "k", 2)

    # ---- program order = scheduler priority.
    # Prelude (DMA-paced; dummies keep the PE dense and preload the exp
    # table): all four c2=0 Q^T/K^T groups + V[0,1]. Keeping the f1 groups
    # here (under the input-DMA window) means the first two attention pairs
    # carry no projection fillers at all.
    pre_t = rc_pool.tile([128, 4], f32, name="pre", tag="pre", bufs=1)
    nc.scalar.activation(pre_t[:], bqk_t[:], AF.Exp, scale=1.0)
    warm(48)
    qkt_sp(qt_t, "q", 0, 0, 0, 0, interleave=2)
    qkt_sp(qt_t, "q", 0, 0, 0, 1, interleave=1)
    dma_wf1()
    qkt_sp(kt_t, "k", 2, 0, 0, 0)
    qkt_sp(kt_t, "k", 2, 0, 0, 1)
    dma_xc21()
    for s in range(8):
        v_group(s)

    # pair(0,1,0): V[3..7] one ki ahead of use; f1/c2=0 (pair(2,3,0)'s
    # scores inputs). At most ONE psum-allocating filler per iteration:
    # a second one serializes behind the exp slot chain. Finishes are
    # emitted incrementally as pv columns stop accumulating.
    p01_0 = (AttnUnit(0, 0), AttnUnit(1, 0))
    p23_0 = (AttnUnit(2, 0), AttnUnit(3, 0))
    attn_pair(p01_0, head=p23_0, fillers=[
        [partial(q_sp, 1, 0, 0)],
        [partial(q_sp, 1, 0, 1)],
        [partial(k_sp, 1, 0, 0)],
        [partial(v_group, 8)],
        [partial(v_group, 9)],
        [fin(p01_0, 0, 512), partial(v_group, 10)],
        [partial(k_sp, 1, 0, 1), partial(v_group, 11)],
        [fin(p01_0, 512, 768), partial(v_group, 12)],
    ])
    fin(p01_0, 768, 896)()
    dma_wp()
    fin(p01_0, 896, 1024)()

    # pair(2,3,0): V[8..13] for the half-1 pairs; f0/c2=1 Q^T (needed by
    # pair(0,1,1)'s scores from its ki 0).
    p01_1 = (AttnUnit(0, 1), AttnUnit(1, 1))
    attn_pair(p23_0, start_ki=1, head=p01_1, fillers=[
        [],
        [partial(v_group, 13)],
        [partial(v_group, 14)],
        [partial(v_group, 15)],
        [partial(q_sp, 0, 1, 0)],
        [fin(p23_0, 0, 512), partial(q_sp, 0, 1, 1)],
        [paw(p23_0, 6)],
        [fin(p23_0, 512, 768), paw(p23_0, 5)],
    ])
    fin(p23_0, 768, 896)()
    fin(p23_0, 896, 1024)()

    # pair(0,1,1): V[13..15], kt f0/c2=1 for its own ki>=8, f1/c2=1 Q^T for
    # pair(2,3,1), then proj(0..3) of the finished half 0 (one half-width
    # group per iteration).
    p23_1 = (AttnUnit(2, 1), AttnUnit(3, 1))
    attn_pair(p01_1, start_ki=1, head=p23_1, fillers=[
        [],
        [partial(k_sp, 0, 1, 0)],
        [partial(k_sp, 0, 1, 1)],
        [partial(q_sp, 1, 1, 0)],
        [partial(q_sp, 1, 1, 1)],
        [partial(proj_h, 0, 0)],
        [partial(proj_h, 0, 1)],
        [partial(proj_h, 1, 0)],
        [partial(proj_h, 1, 1)],
        [partial(proj_h, 2, 0)],
        [partial(proj_h, 2, 1)],
        [paw(p01_1, 6)],
        [paw(p01_1, 6)],
        [fin(p01_1, 0, 512), partial(proj_h, 3, 0), paw(p01_1, 3)],
        [fin(p01_1, 512, 640), partial(proj_h, 3, 1), paw(p01_1, 3)],
        [fin(p01_1, 640, 768), paw(p01_1, 5)],
    ])
    # must precede pair(2,3,1): its pv slots are released by these finishes
    fin(p01_1, 768, 896)()
    fin(p01_1, 896, 1024)()

    # pair(2,3,1): proj(4..7) + its own kt f1/c2=1; finishes emitted at
    # 128-col granularity as psum columns stop accumulating (pv(j) is
    # emitted at loop index j+1); proj(>=8) needs this pair's finishes so
    # it starts at index 13.
    attn_pair(p23_1, start_ki=1, fillers=[
        [],
        [partial(k_sp, 1, 1, 0)],
        [partial(k_sp, 1, 1, 1)],
        [partial(proj_h, 4, 0)],
        [partial(proj_h, 4, 1)],
        [partial(proj_h, 5, 0)],
        [partial(proj_h, 5, 1)],
        [partial(proj_h, 6, 0)],
        [partial(proj_h, 6, 1)],
        [partial(proj_h, 7, 0)],
        [partial(proj_h, 7, 1)],
        [paw(p23_1, 6)],
        [paw(p23_1, 6)],
        [fin(p23_1, 0, 512), partial(proj_h, 8, 0), paw(p23_1, 3)],
        [fin(p23_1, 512, 640), partial(proj_h, 8, 1), partial(proj_h, 9, 0)],
        [fin(p23_1, 640, 768), partial(proj_h, 9, 1), partial(proj_h, 10, 0)],
    ])

    # tail: proj 10,11 need only the 0:512 finish (already done); proj 12 is
    # covered by the 512:768 finishes; the last two finishes overlap the
    # ready projections on the PE. lw() keeps the clock warm to the end.
    fin(p23_1, 768, 896)()
    proj_h(10, 1)
    proj_full(11)
    proj_full(12)
    fin(p23_1, 896, 1024)()
    proj_full(13)
    paw(p23_1, 8)()
    proj_full(14)
    proj_full(15)
    paw(p23_1, 8)()


def _in_maps(x, W_qkv, b_qkv, W_proj):
    bf = ml_dtypes.bfloat16
    maps = []
    for core in range(NCORES):
        b, hg = core // 4, core % 4
        q = W_qkv[:, hg * FPC:(hg + 1) * FPC]
        k = W_qkv[:, D + hg * FPC: D + (hg + 1) * FPC]
        v = W_qkv[:, 2 * D + hg * FPC: 2 * D + (hg + 1) * FPC]
        bq = b_qkv[hg * FPC:(hg + 1) * FPC].astype(np.float32)
        bk = b_qkv[D + hg * FPC: D + (hg + 1) * FPC].astype(np.float32)
        # column order per k-block row group: [q_f0 k_f0 | q_f1 k_f1 | v]
        wpack = np.concatenate(
            [q[:, 0:128], k[:, 0:128], q[:, 128:256], k[:, 128:256], v], axis=1)
        maps.append({
            "xT": np.ascontiguousarray(x[b].T).astype(bf),
            "wqkv": np.ascontiguousarray(wpack).astype(bf),
            "wp": np.ascontiguousarray(W_proj[hg * FPC:(hg + 1) * FPC, :]).astype(bf),
            "bqk": np.ascontiguousarray(
                np.stack([bq[0:128], bq[128:256], bk[0:128], bk[128:256]], axis=1)),
        })
    return maps


def get_nc():
    if "nc" not in _CACHE:
        _CACHE["nc"] = _build()
    return _CACHE["nc"]


def _postprocess(partials, b_qkv, W_proj, b_proj):
    out = np.zeros((B, S, D), np.float32)
    for core in range(NCORES):
        out[core // 4] += np.asarray(partials[core], dtype=np.float32)
    bv = np.asarray(b_qkv, np.float32)[2 * D:3 * D]
    out += bv @ np.asarray(W_proj, np.float32) + np.asarray(b_proj, np.float32)
    return out


def kernel(x, W_qkv, b_qkv, W_proj, b_proj, _trace=False):
    from concourse.bass_utils import run_bass_kernel_spmd

    x = np.asarray(x, np.float32)
    W_qkv = np.asarray(W_qkv, np.float32)
    b_qkv = np.asarray(b_qkv, np.float32)
    W_proj = np.asarray(W_proj, np.float32)
    b_proj = np.asarray(b_proj, np.float32)

    nc = get_nc()
    maps = _in_maps(x, W_qkv, b_qkv, W_proj)
    res = run_bass_kernel_spmd(nc, maps, list(range(NCORES)), trace=_trace)
    _CACHE["last_result"] = res
    partials = [res.results[c]["out"] for c in range(NCORES)]
    return _postprocess(partials, b_qkv, W_proj, b_proj)
